# revision 9
# baseline (speedup 1.0000x reference)
"""Trainium2 Bass kernel for nn_MessagePassingLayer (GNN message passing).

Strategy (8-core SPMD, no collectives):
- Shard by contiguous RECEIVER node ranges (receivers are sorted): core k owns
  nodes [k*N/8, (k+1)*N/8) and the contiguous edge range targeting them.
- Host pre-processing: pads each 128-node window's edge list to a fixed tile
  count, casts/transposes edge features to fp16 feature-major, builds compacted
  per-segment sender tables (int16 gather indices), local receiver indices, and
  window-relative receiver ids for on-chip selection-matrix build.
- On chip per core: stream 512-edge superblocks; dma_gather senders (DRAM fp16,
  transposed) + receivers (SBUF fp16 slice table, transposed); edge MLP in fp16
  matmuls (fp32 PSUM accumulate); segment-sum via per-tile selection-matrix
  matmul accumulating into per-window PSUM; update MLP per window; store f32.
"""
import numpy as np

P = 128


# ---------------------------------------------------------------- host prep

def _pack_idx16(idx, pad_to):
    """[E] -> [128, pad_to//16] int16 in the dma_gather interleaved layout."""
    a = np.full(pad_to, 0, np.int64)
    a[: idx.shape[0]] = idx
    s = a.reshape(pad_to // 16, 16).T.astype(np.uint16).view(np.int16)  # [16, S]
    return np.tile(s, (8, 1))


def host_prep(nodes, edges, senders, receivers, n_cores):
    N, F = nodes.shape
    E = edges.shape[0]
    npc = N // n_cores            # nodes per core
    nwin = -(-npc // P)           # windows per core
    npad = nwin * P

    nodes16 = nodes.astype(np.float16)
    # per-core window boundaries (cores start at multiples of npc, not of P;
    # last window of each core is partial)
    core_bounds = []
    tpw = 1
    for k in range(n_cores):
        base = k * npc
        marks = np.minimum(base + np.arange(nwin + 1) * P, base + npc)
        b = np.searchsorted(receivers, marks)
        core_bounds.append(b)
        tpw = max(tpw, int(-(-(b[1:] - b[:-1]).max() // P)))
    tiles_w = nwin * tpw
    gcalls = -(-(tiles_w * P) // 512)
    tiles = gcalls * 4
    epad = tiles * P

    per_core = []
    max_seg_rows = 0
    for k in range(n_cores):
        base = k * npc
        eT = np.zeros((P, epad), np.float16)
        sidx_flat = np.zeros(epad, np.int64)
        ridx_flat = np.zeros(epad, np.int64)
        rrel = np.full((P, tiles), -1.0, np.float32)
        counts = np.zeros(npad, np.float32)

        for w in range(nwin):
            e0, e1 = core_bounds[k][w], core_bounds[k][w + 1]
            cnt = e1 - e0
            dst0 = w * tpw * P
            eT[:, dst0:dst0 + cnt] = edges[e0:e1].T.astype(np.float16)
            sidx_flat[dst0:dst0 + cnt] = senders[e0:e1]
            ridx_flat[dst0:dst0 + cnt] = receivers[e0:e1] - base
            rr = np.full(tpw * P, -1.0, np.float32)
            rr[:cnt] = (receivers[e0:e1] - base - w * P).astype(np.float32)
            rrel[:, w * tpw:(w + 1) * tpw] = rr.reshape(tpw, P).T
            nb = np.bincount(receivers[e0:e1] - base - w * P, minlength=P)
            counts[w * P:(w + 1) * P] = nb

        per_core.append(dict(base=base, eT=eT, sidx_flat=sidx_flat,
                             ridx_flat=ridx_flat, rrel=rrel, counts=counts))

    # choose segments so each segment's distinct senders fit int16
    segments = 1
    while True:
        segcalls = -(-gcalls // segments)
        worst = 0
        for pc in per_core:
            for s in range(segments):
                c0, c1 = s * segcalls, min((s + 1) * segcalls, gcalls)
                if c0 >= c1:
                    continue
                u = np.unique(pc["sidx_flat"][c0 * 512:c1 * 512])
                worst = max(worst, u.shape[0])
        if worst <= 30000 or segments >= 16:
            break
        segments *= 2
    seg_rows = -(-worst // P) * P
    segcalls = -(-gcalls // segments)

    for pc in per_core:
        stab = np.zeros((segments * seg_rows, F), np.float16)
        sloc = np.zeros(epad, np.int64)
        for s in range(segments):
            c0, c1 = s * segcalls, min((s + 1) * segcalls, gcalls)
            if c0 >= c1:
                continue
            sl = slice(c0 * 512, c1 * 512)
            u, inv = np.unique(pc["sidx_flat"][sl], return_inverse=True)
            stab[s * seg_rows:s * seg_rows + u.shape[0]] = nodes16[u]
            sloc[sl] = inv
        rtab = np.zeros((npad, F), np.float16)
        lo = min(pc["base"] + npad, N) - pc["base"]
        rtab[:lo] = nodes16[pc["base"]:pc["base"] + lo]
        nodesT = np.zeros((P, npad), np.float16)
        nodesT[:, :lo] = nodes16[pc["base"]:pc["base"] + lo].T
        pc.update(stab=stab,
                  sidx=_pack_idx16(sloc, epad),
                  ridx=_pack_idx16(pc["ridx_flat"], epad),
                  rtab=rtab, nodesT=nodesT)

    meta = dict(npc=npc, nwin=nwin, npad=npad, tpw=tpw, tiles_w=tiles_w,
                gcalls=gcalls, tiles=tiles, epad=epad,
                segments=segments, segcalls=segcalls, seg_rows=seg_rows)
    return per_core, meta


# ---------------------------------------------------------------- device build

def build_nc(meta, n_cores, act_mode="silu"):
    import concourse.bacc as bacc
    import concourse.tile as tile
    from concourse import mybir, library_config

    F = H = 128
    npad, nwin, tpw = meta["npad"], meta["nwin"], meta["tpw"]
    gcalls, tiles_w = meta["gcalls"], meta["tiles_w"]
    epad, segments, segcalls, seg_rows = (
        meta["epad"], meta["segments"], meta["segcalls"], meta["seg_rows"])
    fp16, f32 = mybir.dt.float16, mybir.dt.float32

    nc = bacc.Bacc("TRN2", target_bir_lowering=False, debug=False,
                   num_devices=n_cores)
    D = lambda n, s, t, k: nc.dram_tensor(n, s, t, kind=k).ap()
    edgesT = D("edgesT", [P, epad], fp16, "ExternalInput")
    stab = D("stab", [segments * seg_rows, F], fp16, "ExternalInput")
    sidx = D("sidx", [P, epad // 16], mybir.dt.int16, "ExternalInput")
    rtab = D("rtab", [npad, F], fp16, "ExternalInput")
    ridx = D("ridx", [P, epad // 16], mybir.dt.int16, "ExternalInput")
    rrel = D("rrel", [P, meta["tiles"]], f32, "ExternalInput")
    nodesT = D("nodesT", [P, npad], fp16, "ExternalInput")
    counts = D("counts", [1, npad], fp16, "ExternalInput")
    iota = D("iota", [P, P], fp16, "ExternalInput")
    wm = D("wm", [P, 4 * P], fp16, "ExternalInput")      # Wm1s|Wm1r|Wm1e|Wm2
    wu = D("wu", [P, 3 * P], fp16, "ExternalInput")      # Wu1F|Wu1H|Wu2
    bm1 = D("bm1", [P, 1], f32, "ExternalInput")
    bu1 = D("bu1", [P, 1], f32, "ExternalInput")
    bm2 = D("bm2", [1, P], fp16, "ExternalInput")
    bu2b = D("bu2b", [P, P], f32, "ExternalInput")
    out = D("out", [npad, P], f32, "ExternalOutput")

    SILU = mybir.ActivationFunctionType.Silu
    EQ = mybir.AluOpType.is_equal

    with tile.TileContext(nc) as tc:
        with (tc.tile_pool(name="const", bufs=1) as cp,
              tc.tile_pool(name="eg", bufs=3) as egp,
              tc.tile_pool(name="sg", bufs=3) as sgp,
              tc.tile_pool(name="rg", bufs=3) as rgp,
              tc.tile_pool(name="h1", bufs=2) as h1p,
              tc.tile_pool(name="ms", bufs=2) as msp,
              tc.tile_pool(name="sm", bufs=6) as smp,
              tc.tile_pool(name="wsb", bufs=2) as wsbp,
              tc.tile_pool(name="osb", bufs=2) as osbp,
              tc.tile_pool(name="l1ps", bufs=2, space="PSUM") as l1ps,
              tc.tile_pool(name="msps", bufs=2, space="PSUM") as msps,
              tc.tile_pool(name="aggps", bufs=2, space="PSUM") as aggps,
              tc.tile_pool(name="ups", bufs=2, space="PSUM") as ups):
            nc.gpsimd.load_library(library_config.mlp)

            # ---- resident loads
            sidx_sb = cp.tile([P, epad // 16], mybir.dt.int16)
            nc.sync.dma_start(sidx_sb[:], sidx)
            ridx_sb = cp.tile([P, epad // 16], mybir.dt.int16)
            nc.sync.dma_start(ridx_sb[:], ridx)
            rtab_sb = cp.tile([P, npad], fp16)
            nc.sync.dma_start(rtab_sb[:].rearrange("p (b f) -> p b f", f=F),
                              rtab.rearrange("(b p) f -> p b f", p=P))
            rrel_sb = cp.tile([P, meta["tiles"]], f32)
            nc.sync.dma_start(rrel_sb[:], rrel)
            nodesT_sb = cp.tile([P, npad], fp16)
            nc.sync.dma_start(nodesT_sb[:], nodesT)
            counts_sb = cp.tile([1, npad], fp16)
            nc.sync.dma_start(counts_sb[:], counts)
            iota_sb = cp.tile([P, P], fp16)
            nc.sync.dma_start(iota_sb[:], iota)
            wm_sb = cp.tile([P, 4 * P], fp16)
            nc.sync.dma_start(wm_sb[:], wm)
            wu_sb = cp.tile([P, 3 * P], fp16)
            nc.sync.dma_start(wu_sb[:], wu)
            bm1_sb = cp.tile([P, 1], f32)
            nc.sync.dma_start(bm1_sb[:], bm1)
            bu1_sb = cp.tile([P, 1], f32)
            nc.sync.dma_start(bu1_sb[:], bu1)
            bm2_sb = cp.tile([1, P], fp16)
            nc.sync.dma_start(bm2_sb[:], bm2)
            bu2b_sb = cp.tile([P, P], f32)
            nc.sync.dma_start(bu2b_sb[:], bu2b)

            wms = lambda i: wm_sb[:, i * P:(i + 1) * P]
            agg_live = {}

            for c in range(gcalls):
                seg = min(c // segcalls, segments - 1)
                sg = sgp.tile([P, 512], fp16, tag="sg")
                nc.gpsimd.dma_gather(
                    out_ap=sg[:].unsqueeze(1),
                    in_ap=stab[seg * seg_rows:(seg + 1) * seg_rows, :],
                    idxs_ap=sidx_sb[:, c * 32:(c + 1) * 32],
                    num_idxs=512, num_idxs_reg=512, elem_size=F, transpose=True)
                rg = rgp.tile([P, 512], fp16, tag="rg")
                nc.gpsimd.dma_gather(
                    out_ap=rg[:].unsqueeze(1), in_ap=rtab_sb[:],
                    idxs_ap=ridx_sb[:, c * 32:(c + 1) * 32],
                    num_idxs=512, num_idxs_reg=512, elem_size=F, transpose=True,
                    sbuf_tokens_per_rank=P, sbuf_free_dim_per_rank=F * 2)
                eg = egp.tile([P, 512], fp16, tag="eg")
                nc.sync.dma_start(eg[:], edgesT[:, c * 512:(c + 1) * 512])

                l1 = l1ps.tile([P, 512], f32, tag="l1")
                nc.tensor.matmul(l1[:], wms(0), sg[:], start=True, stop=False)
                nc.tensor.matmul(l1[:], wms(1), rg[:], start=False, stop=False)
                nc.tensor.matmul(l1[:], wms(2), eg[:], start=False, stop=True)
                h1s = h1p.tile([P, 512], fp16, tag="h1")
                if act_mode == "silu":
                    nc.scalar.activation(h1s[:], l1[:], SILU, bias=bm1_sb[:])
                else:
                    ssig = h1p.tile([P, 512], fp16, tag="ssig")
                    nc.scalar.activation(ssig[:], l1[:],
                                         mybir.ActivationFunctionType.Sigmoid)
                    nc.vector.tensor_tensor(out=h1s[:], in0=ssig[:], in1=l1[:],
                                            op=mybir.AluOpType.mult)

                mp = msps.tile([P, 512], f32, tag="ms")
                for t4 in range(4):
                    nc.tensor.matmul(mp[:, t4 * P:(t4 + 1) * P],
                                     h1s[:, t4 * P:(t4 + 1) * P], wms(3),
                                     start=True, stop=True)
                msgs = msp.tile([P, 512], fp16, tag="msgs")
                nc.vector.tensor_copy(msgs[:], mp[:])

                for t4 in range(4):
                    t = c * 4 + t4
                    if t >= tiles_w:
                        continue
                    w = t // tpw
                    S = smp.tile([P, P], fp16, tag="S")
                    nc.vector.tensor_scalar(
                        out=S[:], in0=iota_sb[:], scalar1=rrel_sb[:, t:t + 1],
                        scalar2=None, op0=EQ)
                    first, last = t % tpw == 0, t % tpw == tpw - 1
                    if first:
                        agg_t = aggps.tile([P, P], f32, tag="agg")
                        agg_live[w] = agg_t
                    nc.tensor.matmul(agg_live[w][:], msgs[:, t4 * P:(t4 + 1) * P],
                                     S[:], start=first, stop=False)
                    if last:
                        agg = agg_live.pop(w)
                        nc.tensor.matmul(agg[:], bm2_sb[:],
                                         counts_sb[0:1, w * P:(w + 1) * P],
                                         start=False, stop=True)
                        aggsb = wsbp.tile([P, P], fp16, tag="aggsb")
                        nc.vector.tensor_copy(aggsb[:], agg[:])
                        hu = ups.tile([P, P], f32, tag="u")
                        nc.tensor.matmul(hu[:], wu_sb[:, 0:P],
                                         nodesT_sb[:, w * P:(w + 1) * P],
                                         start=True, stop=False)
                        nc.tensor.matmul(hu[:], wu_sb[:, P:2 * P], aggsb[:],
                                         start=False, stop=True)
                        hus = wsbp.tile([P, P], fp16, tag="hus")
                        if act_mode == "silu":
                            nc.scalar.activation(hus[:], hu[:], SILU,
                                                 bias=bu1_sb[:])
                        else:
                            usig = wsbp.tile([P, P], fp16, tag="usig")
                            nc.scalar.activation(
                                usig[:], hu[:],
                                mybir.ActivationFunctionType.Sigmoid)
                            nc.vector.tensor_tensor(
                                out=hus[:], in0=usig[:], in1=hu[:],
                                op=mybir.AluOpType.mult)
                        uo = ups.tile([P, P], f32, tag="u")
                        nc.tensor.matmul(uo[:], hus[:], wu_sb[:, 2 * P:3 * P],
                                         start=True, stop=True)
                        osb = osbp.tile([P, P], f32, tag="osb")
                        nc.vector.tensor_add(osb[:], uo[:], bu2b_sb[:])
                        nc.sync.dma_start(out[w * P:(w + 1) * P, :], osb[:])
    nc.compile()
    return nc


# ---------------------------------------------------------------- entry point

_CACHE = {}


def _build_in_maps(inputs, n_cores=8):
    nodes = np.asarray(inputs["nodes"], np.float32)
    edges = np.asarray(inputs["edges"], np.float32)
    senders = np.asarray(inputs["senders"], np.int64)
    receivers = np.asarray(inputs["receivers"], np.int64)
    Wm1 = np.asarray(inputs["Wm1"], np.float32)
    bm1 = np.asarray(inputs["bm1"], np.float32)
    Wm2 = np.asarray(inputs["Wm2"], np.float32)
    bm2 = np.asarray(inputs["bm2"], np.float32)
    Wu1 = np.asarray(inputs["Wu1"], np.float32)
    bu1 = np.asarray(inputs["bu1"], np.float32)
    Wu2 = np.asarray(inputs["Wu2"], np.float32)
    bu2 = np.asarray(inputs["bu2"], np.float32)
    N, F = nodes.shape
    H = Wm2.shape[1]

    per_core, meta = host_prep(nodes, edges, senders, receivers, n_cores)
    wm_h = np.concatenate([Wm1[0:F], Wm1[F:2 * F], Wm1[2 * F:3 * F], Wm2],
                          axis=1).astype(np.float16)
    wu_h = np.concatenate([Wu1[0:F], Wu1[F:F + H], Wu2], axis=1).astype(np.float16)
    iota_h = np.tile(np.arange(P, dtype=np.float16)[None, :], (P, 1))
    in_maps = []
    for pc in per_core:
        in_maps.append(dict(
            edgesT=pc["eT"], stab=pc["stab"], sidx=pc["sidx"],
            rtab=pc["rtab"], ridx=pc["ridx"], rrel=pc["rrel"],
            nodesT=pc["nodesT"], counts=pc["counts"].astype(np.float16)[None, :],
            iota=iota_h, wm=wm_h, wu=wu_h,
            bm1=bm1.reshape(P, 1), bu1=bu1.reshape(P, 1),
            bm2=bm2.astype(np.float16).reshape(1, P),
            bu2b=np.tile(bu2[None, :], (P, 1)).astype(np.float32)))
    return in_maps, meta, per_core


def kernel(**inputs):
    from concourse.bass_utils import run_bass_kernel_spmd
    n_cores = 8
    nodes = np.asarray(inputs["nodes"], np.float32)
    N = nodes.shape[0]
    in_maps, meta, per_core = _build_in_maps(inputs, n_cores)
    key = (N, meta["tpw"], meta["epad"], meta["segments"], meta["seg_rows"])
    if key not in _CACHE:
        _CACHE[key] = build_nc(meta, n_cores)
    nc = _CACHE[key]
    res = run_bass_kernel_spmd(nc, in_maps, list(range(n_cores)))
    npc = meta["npc"]
    out = np.empty((N, 128), np.float32)
    for k in range(n_cores):
        out[k * npc:(k + 1) * npc] = res.results[k]["out"][:npc]
    return out
